# revision 9
# baseline (speedup 1.0000x reference)
"""GCN 2-layer encoder on 8 TRN2 NeuronCores — v2.

Aggregate-first form per core (nodes sharded):
  out = A_hat @ (relu((A_hat @ x) @ W1 + b1) @ W2) + b2

v2 layout/scheduling changes vs v1:
  - Gather calls batched: one dma_gather per (group of G dst tiles, bank)
    -> 4*ceil(TD/G) calls/layer instead of 4*TD. Per-(tile,bank) subtile
    counts T[t,b] sized to the max run across the 8 cores.
  - One shared int16 index table + one msgs SBUF pool for both layers
    (layer-2 gathers write into a prefix view of the same tiles).
  - Selector built per subtile via DVE tensor_scalar
    (iota == dstl_col) * dinvslot_col  -> hits the 2x/4x DVE path and
    folds the dinv[dst] epilogue scale into the segsum matmul.
  - Node remap: core c owns global rows [c*HC,(c+1)*HC) u
    [HALF+c*HC, HALF+(c+1)*HC), HC=NPAD/16, so the h2 AllGather splits
    into two contiguous-output chunks; chunk 0 is issued after the
    first half of layer 1 and overlaps the rest of layer 1.
"""
import sys

sys.path.insert(0, "/opt/trn_rl_repo")
import os
import numpy as np
import ml_dtypes

import concourse.bass as bass
import concourse.bacc as bacc
import concourse.mybir as mybir
import concourse.tile as tile
from concourse import bass_utils
from concourse.masks import make_identity

P = 128
NC = 8
N = 100_000
NPAD = 100_352
SHARD = NPAD // NC  # 12544
TD = SHARD // P  # 98 dst tiles per core
NB = 4
BANK = NPAD // NB  # 25088
HC = SHARD // 2  # 6272 rows per shard chunk
HALF = NPAD // 2  # 50176
G = 7  # dst tiles per gather group
NG = TD // G  # 14 groups
D_IN = 256
H1 = 256
H2 = 128
f32 = mybir.dt.float32
f32r = mybir.dt.float32r
bf16 = mybir.dt.bfloat16
i16 = mybir.dt.int16

LAST_EXEC_NS = None
LAST_RESULT = None
LAST_TB = None


def _remap_ids(o):
    """old node id -> new node id (chunked shards for 2-piece AllGather)."""
    c = o // SHARD
    r = o % SHARD
    lo = r < HC
    return np.where(lo, c * HC + r, HALF + c * HC + (r - HC))


def _pack(edge_src, edge_dst, dinv_pad):
    """Sort edges (already in remapped id space) into per-core slot layouts.

    Layout: for grp in NG: for b in NB: for t in grp: T[t,b]*128 slots.
    Returns T table and per-core arrays (gidx wrapped, dstl cols,
    dinvslot cols).
    """
    percore = []
    runs_all = np.zeros((NC, TD, NB), dtype=np.int64)
    for c in range(NC):
        lo, hi = c * SHARD, (c + 1) * SHARD
        # new-id shard of core c is two chunks; map dst new-id -> local row
        # local row: chunk0 -> r, chunk1 -> HC + r
        d = edge_dst
        in0 = (d >= c * HC) & (d < (c + 1) * HC)
        in1 = (d >= HALF + c * HC) & (d < HALF + (c + 1) * HC)
        sel = in0 | in1
        s, d = edge_src[sel], edge_dst[sel]
        local = np.where(d < HALF, d - c * HC, HC + (d - (HALF + c * HC)))
        tile_id = local // P
        bank = s // BANK
        key = tile_id * NB + bank
        order = np.argsort(key, kind="stable")
        s, local, key = s[order], local[order], key[order]
        runs = np.bincount(key, minlength=TD * NB).reshape(TD, NB)
        runs_all[c] = runs
        percore.append((s, local, key, runs))
    T = (runs_all.max(axis=0) + P - 1) // P  # [TD, NB]
    T = np.maximum(T, 1)
    # slot base for each (t, b) in the global layout
    slot_base = np.zeros((TD, NB), dtype=np.int64)
    call_base = []  # (grp, b) -> slot offset, num_idxs
    off = 0
    for g in range(NG):
        for b in range(NB):
            call_off = off
            for t in range(g * G, (g + 1) * G):
                slot_base[t, b] = off
                off += T[t, b] * P
            call_base.append((call_off, off - call_off))
    nslots = off
    nsub = int(T.sum())
    # subtile column index for (t,b,j): order must match device loop:
    # for grp: for t in grp: for b: for j  -> column order chosen here
    subcol = np.zeros((TD, NB), dtype=np.int64)
    col = 0
    for g in range(NG):
        for t in range(g * G, (g + 1) * G):
            for b in range(NB):
                subcol[t, b] = col
                col += T[t, b]
    assert col == nsub

    cores = []
    for c in range(NC):
        s, local, key, runs = percore[c]
        first = np.zeros(TD * NB, dtype=np.int64)
        first[1:] = np.cumsum(runs.reshape(-1))[:-1]
        rank = np.arange(len(key)) - first[key]
        slot = slot_base[key // NB, key % NB] + rank
        gidx = np.zeros(nslots, dtype=np.int16)
        dstl = np.full(nslots, -1.0, dtype=np.float32)
        dslot = np.zeros(nslots, dtype=np.float32)
        gidx[slot] = (s % BANK).astype(np.int16)
        dstl[slot] = (local % P).astype(np.float32)
        # dinv of dst node (new-id space)
        gdst = np.where(local < HC, c * HC + local, HALF + c * HC + (local - HC))
        dslot[slot] = dinv_pad[gdst]
        # wrap idxs per call
        parts = []
        for g in range(NG):
            for b in range(NB):
                co, n = call_base[g * NB + b]
                parts.append(gidx[co:co + n].reshape(n // 16, 16).T)
        wrapped = np.tile(np.concatenate(parts, axis=1), (8, 1))
        # dstl/dinvslot as [128, nsub] column tables in subcol order
        dstl_cols = np.empty((P, nsub), dtype=np.float32)
        dslot_cols = np.empty((P, nsub), dtype=np.float32)
        for t in range(TD):
            for b in range(NB):
                c0 = subcol[t, b]
                nT = T[t, b]
                blk = dstl[slot_base[t, b]:slot_base[t, b] + nT * P]
                dstl_cols[:, c0:c0 + nT] = blk.reshape(nT, P).T
                blk2 = dslot[slot_base[t, b]:slot_base[t, b] + nT * P]
                dslot_cols[:, c0:c0 + nT] = blk2.reshape(nT, P).T
        cores.append({
            "gidx": np.ascontiguousarray(wrapped),
            "dstl": dstl_cols,
            "dslot": dslot_cols,
        })
    return T, call_base, subcol, nslots, nsub, cores


def _build(Tkey, T, call_base, subcol, nslots, nsub):
    TMAXC = max(n for _, n in call_base) // P  # max subtiles per call
    nc = bacc.Bacc(
        "TRN2",
        target_bir_lowering=False,
        debug=False,
        num_devices=NC,
        num_swdge_queues=4,
    )
    xs = nc.dram_tensor("xs", [NPAD, D_IN], bf16, kind="ExternalInput").ap()
    gidx = nc.dram_tensor("gidx", [P, nslots // 16], i16, kind="ExternalInput").ap()
    dstl_d = nc.dram_tensor("dstl", [P, nsub], f32, kind="ExternalInput").ap()
    dslot_d = nc.dram_tensor("dslot", [P, nsub], f32, kind="ExternalInput").ap()
    w1 = nc.dram_tensor("w1", [D_IN, H1], f32, kind="ExternalInput").ap()
    w2 = nc.dram_tensor("w2", [H1, H2], f32, kind="ExternalInput").ap()
    b1c = nc.dram_tensor("b1c", [P, H1 // P], f32, kind="ExternalInput").ap()
    b2r = nc.dram_tensor("b2r", [P, H2], f32, kind="ExternalInput").ap()
    dinv_d = nc.dram_tensor("dinv_d", [P, TD], f32, kind="ExternalInput").ap()
    out = nc.dram_tensor("out", [SHARD, H2], f32, kind="ExternalOutput").ap()

    qn = [0]

    def next_q():
        qn[0] = (qn[0] + 1) % 4
        return qn[0]

    with tile.TileContext(nc) as tc:
        with (
            tc.tile_pool(name="const", bufs=1) as cp,
            tc.tile_pool(name="msg", bufs=8) as mp,
            tc.tile_pool(name="sel", bufs=8) as sp,
            tc.tile_pool(name="work", bufs=2) as wp,
            tc.tile_pool(name="grp", bufs=2) as gp,
            tc.tile_pool(name="psy", bufs=2, space="PSUM") as psy,
            tc.tile_pool(name="pst", bufs=2, space="PSUM") as pst,
            tc.tile_pool(name="psh", bufs=2, space="PSUM") as psh,
            tc.tile_pool(name="dram", bufs=1, space="DRAM") as dp,
        ):
            # ---- constants ----
            iota_i = cp.tile([P, P], mybir.dt.int32)
            nc.gpsimd.iota(iota_i[:], pattern=[[1, P]], base=0, channel_multiplier=0)
            iota_bf = cp.tile([P, P], bf16)
            nc.vector.tensor_copy(iota_bf[:], iota_i[:])
            ident = cp.tile([P, P], f32)
            make_identity(nc, ident[:])

            gidx_t = cp.tile([P, nslots // 16], i16)
            nc.sync.dma_start(gidx_t[:], gidx[:, :])
            dstl_t = cp.tile([P, nsub], f32)
            nc.sync.dma_start(dstl_t[:], dstl_d[:, :])
            dslot_t = cp.tile([P, nsub], f32)
            nc.sync.dma_start(dslot_t[:], dslot_d[:, :])
            b1_t = cp.tile([P, H1 // P], f32)
            nc.sync.dma_start(b1_t[:], b1c[:, :])
            b2_t = cp.tile([P, H2], f32)
            nc.sync.dma_start(b2_t[:], b2r[:, :])
            dinv_t = cp.tile([P, TD], f32)
            nc.sync.dma_start(dinv_t[:], dinv_d[:, :])
            w1_t = [cp.tile([P, H1], f32r, tag=f"w1_{k}", name=f"w1_{k}") for k in range(2)]
            for k in range(2):
                nc.gpsimd.dma_start(w1_t[k][:], w1[k * P:(k + 1) * P, :])
            w2_t = [cp.tile([P, H2], f32r, tag=f"w2_{k}", name=f"w2_{k}") for k in range(2)]
            for k in range(2):
                nc.gpsimd.dma_start(w2_t[k][:], w2[k * P:(k + 1) * P, :])

            h2sA = dp.tile([HC, H2], bf16)
            h2sB = dp.tile([HC, H2], bf16)
            h2A = dp.tile([HALF, H2], bf16)
            h2B = dp.tile([HALF, H2], bf16)

            def gather_call(g, b, elem, src_ap):
                co, n = call_base[g * NB + b]
                m = mp.tile([P, TMAXC, D_IN], bf16, tag="msg", name="m")
                nsubt = n // P
                # view the first nsubt*elem columns as [P, nsubt, elem]
                flat = m[:].rearrange("p t d -> p (t d)")
                view = flat[:, :nsubt * elem].rearrange("p (t d) -> p t d", t=nsubt)
                nc.gpsimd.dma_gather(
                    out_ap=view,
                    in_ap=src_ap,
                    idxs_ap=gidx_t[:, co // 16:(co + n) // 16],
                    num_idxs=n,
                    num_idxs_reg=n,
                    elem_size=elem,
                    single_packet=False,
                    queue_num=next_q(),
                )
                return m, nsubt

            def segsum(g, t, msgs, elem, acc):
                """acc[dst,elem] += sum over subtiles sel^T @ msg."""
                K = int(T[t].sum())
                k = 0
                for b in range(NB):
                    m = msgs[b]
                    flat = m[:].rearrange("p t d -> p (t d)")
                    # subtile offset of tile t within call (g,b)
                    sb = 0
                    for t2 in range(g * G, t):
                        sb += int(T[t2, b])
                    for j in range(int(T[t, b])):
                        col = int(subcol[t, b]) + j
                        sel = sp.tile([P, P], bf16, tag="sel", name="sel")
                        nc.vector.tensor_scalar(
                            out=sel[:],
                            in0=iota_bf[:],
                            scalar1=dstl_t[:, col:col + 1],
                            scalar2=dslot_t[:, col:col + 1],
                            op0=mybir.AluOpType.is_equal,
                            op1=mybir.AluOpType.mult,
                        )
                        w = sb + j
                        rhs = flat[:, w * elem:(w + 1) * elem]
                        nc.tensor.matmul(
                            acc[:],
                            lhsT=sel[:],
                            rhs=rhs,
                            start=(k == 0),
                            stop=(k == K - 1),
                        )
                        k += 1

            # ================= layer 1 =================
            for g in range(NG):
                msgs = []
                for b in range(NB):
                    m, _ = gather_call(g, b, D_IN, xs[b * BANK:(b + 1) * BANK, :])
                    msgs.append(m)
                yT = [gp.tile([P, G * P], f32r, tag=f"yT{h}", name=f"yT{h}") for h in range(2)]
                for ti in range(G):
                    t = g * G + ti
                    acc = psy.tile([P, D_IN], f32, tag="acc1")
                    segsum(g, t, msgs, D_IN, acc)
                    y_sb = wp.tile([P, D_IN], f32, tag="ysb")
                    nc.scalar.copy(out=y_sb[:], in_=acc[:])
                    for h in range(2):
                        tp = pst.tile([P, P], f32, tag="tp")
                        nc.tensor.transpose(
                            out=tp[:], in_=y_sb[:, h * P:(h + 1) * P], identity=ident[:]
                        )
                        nc.vector.tensor_copy(yT[h][:, ti * P:(ti + 1) * P], tp[:])
                # dense, in PSUM-bank-sized column chunks (<=512 f32)
                nn = G * P
                chunks = [(0, 512), (512, nn)]
                rT = [gp.tile([P, G * P], f32r, tag=f"rT{o}", name=f"rT{o}") for o in range(2)]
                h2T_sb = wp.tile([P, G * P], f32, tag="h2T")
                for c0, c1 in chunks:
                    cw = c1 - c0
                    for o in range(2):
                        ph1 = psh.tile([P, 512], f32, tag="ph1")
                        for k in range(2):
                            nc.tensor.matmul(
                                ph1[:, :cw],
                                lhsT=w1_t[k][:, o * P:(o + 1) * P],
                                rhs=yT[k][:, c0:c1],
                                start=(k == 0),
                                stop=(k == 1),
                            )
                        nc.scalar.activation(
                            out=rT[o][:, c0:c1],
                            in_=ph1[:, :cw],
                            func=mybir.ActivationFunctionType.Relu,
                            bias=b1_t[:, o:o + 1],
                            scale=1.0,
                        )
                    ph2 = psh.tile([P, 512], f32, tag="ph1")
                    for k in range(2):
                        nc.tensor.matmul(
                            ph2[:, :cw],
                            lhsT=w2_t[k][:, :],
                            rhs=rT[k][:, c0:c1],
                            start=(k == 0),
                            stop=(k == 1),
                        )
                    nc.vector.tensor_copy(h2T_sb[:, c0:c1], ph2[:, :cw])
                h2_sb = wp.tile([P, G, P], bf16, tag="h2sb")
                for ti in range(G):
                    t = g * G + ti
                    tp2 = pst.tile([P, P], f32, tag="tp")
                    nc.tensor.transpose(
                        out=tp2[:], in_=h2T_sb[:, ti * P:(ti + 1) * P], identity=ident[:]
                    )
                    nc.vector.tensor_scalar(
                        out=h2_sb[:, ti, :],
                        in0=tp2[:],
                        scalar1=dinv_t[:, t:t + 1],
                        scalar2=None,
                        op0=mybir.AluOpType.mult,
                    )
                if g < NG // 2:
                    dst_rows = h2sA[g * G * P:(g * G + G) * P, :].rearrange(
                        "(t p) f -> p t f", p=P)
                else:
                    g2 = g - NG // 2
                    dst_rows = h2sB[g2 * G * P:(g2 * G + G) * P, :].rearrange(
                        "(t p) f -> p t f", p=P)
                nc.sync.dma_start(dst_rows, h2_sb[:, :, :])

                # chunk-0 AllGather once the first half of the shard is done
                if g == NG // 2 - 1:
                    nc.gpsimd.collective_compute(
                        "AllGather",
                        mybir.AluOpType.bypass,
                        ins=[h2sA.opt()],
                        outs=[h2A.opt()],
                        replica_groups=[list(range(NC))],
                    )
            nc.gpsimd.collective_compute(
                "AllGather",
                mybir.AluOpType.bypass,
                ins=[h2sB.opt()],
                outs=[h2B.opt()],
                replica_groups=[list(range(NC))],
            )

            # ================= layer 2 =================
            for g in range(NG):
                msgs = []
                for b in range(NB):
                    if b < 2:
                        src = h2A[b * BANK:(b + 1) * BANK, :]
                    else:
                        src = h2B[(b - 2) * BANK:(b - 1) * BANK, :]
                    m, _ = gather_call(g, b, H2, src)
                    msgs.append(m)
                for ti in range(G):
                    t = g * G + ti
                    acc2 = pst.tile([P, H2], f32, tag="tp")
                    segsum(g, t, msgs, H2, acc2)
                    o_sb = wp.tile([P, H2], f32, tag="osb")
                    nc.vector.tensor_tensor(
                        out=o_sb[:],
                        in0=acc2[:],
                        in1=b2_t[:],
                        op=mybir.AluOpType.add,
                    )
                    nc.sync.dma_start(out[t * P:(t + 1) * P, :], o_sb[:])

    nc.compile()
    return nc


_CACHED = {}


def kernel(x, W1, b1, W2, b2, edge_index):
    global LAST_EXEC_NS, LAST_RESULT, LAST_TB
    x = np.asarray(x, dtype=np.float32)
    W1 = np.asarray(W1, dtype=np.float32)
    b1 = np.asarray(b1, dtype=np.float32)
    W2 = np.asarray(W2, dtype=np.float32)
    b2 = np.asarray(b2, dtype=np.float32)
    ei = np.asarray(edge_index)
    src = ei[0].astype(np.int64)
    dst = ei[1].astype(np.int64)
    n = x.shape[0]
    loop = np.arange(n, dtype=np.int64)
    src_f = np.concatenate([src, loop])
    dst_f = np.concatenate([dst, loop])
    deg = np.bincount(dst_f, minlength=n).astype(np.float32)
    dinv = np.where(deg > 0, 1.0 / np.sqrt(deg), 0.0).astype(np.float32)

    # remap ids to chunked-shard space
    src_r = _remap_ids(src_f)
    dst_r = _remap_ids(dst_f)
    newpos = _remap_ids(np.arange(NPAD, dtype=np.int64))  # old -> new
    dinv_pad = np.zeros(NPAD, dtype=np.float32)
    dinv_pad[:n] = dinv
    dinv_new = np.zeros(NPAD, dtype=np.float32)
    dinv_new[newpos] = dinv_pad

    xs = np.zeros((NPAD, D_IN), dtype=ml_dtypes.bfloat16)
    xs[newpos[:n]] = (x * dinv[:, None]).astype(ml_dtypes.bfloat16)

    T, call_base, subcol, nslots, nsub, cores = _pack(src_r, dst_r, dinv_new)
    LAST_TB = (nslots, nsub)

    key = hash((T.tobytes(), G))
    if key not in _CACHED:
        _CACHED[key] = _build(key, T, call_base, subcol, nslots, nsub)
    ncobj = _CACHED[key]

    b1c = b1.reshape(H1 // P, P).T.copy()
    b2r = np.tile(b2.reshape(1, H2), (P, 1)).astype(np.float32)
    in_maps = []
    for c in range(NC):
        # dinv per dst tile, new-id order: tiles 0..48 chunk0, 49..97 chunk1
        rows = np.concatenate([
            dinv_new[c * HC:(c + 1) * HC],
            dinv_new[HALF + c * HC:HALF + (c + 1) * HC],
        ])
        dinv_d = rows.reshape(TD, P).T.copy()
        in_maps.append({
            "xs": xs,
            "gidx": cores[c]["gidx"],
            "dstl": cores[c]["dstl"],
            "dslot": cores[c]["dslot"],
            "w1": W1,
            "w2": W2,
            "b1c": b1c,
            "b2r": b2r,
            "dinv_d": dinv_d,
        })

    trace = os.environ.get("KERNEL_TRACE", "0") == "1"
    if trace:
        try:
            import profhook

            profhook.install()
        except Exception:
            trace = False
    res = bass_utils.run_bass_kernel_spmd(
        ncobj, in_maps, core_ids=list(range(NC)), trace=trace
    )
    LAST_EXEC_NS = res.exec_time_ns
    LAST_RESULT = res
    out_new = np.zeros((NPAD, H2), dtype=np.float32)
    for c in range(NC):
        r = np.asarray(res.results[c]["out"], dtype=np.float32)
        out_new[c * HC:(c + 1) * HC] = r[:HC]
        out_new[HALF + c * HC:HALF + (c + 1) * HC] = r[HC:]
    out = out_new[newpos[:n]]
    return out.astype(np.float32)


# revision 21
# speedup vs baseline: 1.0332x; 1.0332x over previous
"""GCN 2-layer encoder on 8 TRN2 NeuronCores — v2.

Aggregate-first form per core (nodes sharded):
  out = A_hat @ (relu((A_hat @ x) @ W1 + b1) @ W2) + b2

v2 layout/scheduling changes vs v1:
  - Gather calls batched: one dma_gather per (group of G dst tiles, bank)
    -> 4*ceil(TD/G) calls/layer instead of 4*TD. Per-(tile,bank) subtile
    counts T[t,b] sized to the max run across the 8 cores.
  - One shared int16 index table + one msgs SBUF pool for both layers
    (layer-2 gathers write into a prefix view of the same tiles).
  - Selector built per subtile via DVE tensor_scalar
    (iota == dstl_col) * dinvslot_col  -> hits the 2x/4x DVE path and
    folds the dinv[dst] epilogue scale into the segsum matmul.
  - Node remap: core c owns global rows [c*HC,(c+1)*HC) u
    [HALF+c*HC, HALF+(c+1)*HC), HC=NPAD/16, so the h2 AllGather splits
    into two contiguous-output chunks; chunk 0 is issued after the
    first half of layer 1 and overlaps the rest of layer 1.
"""
import sys

sys.path.insert(0, "/opt/trn_rl_repo")
import os
import numpy as np
import ml_dtypes

import concourse.bass as bass
import concourse.bacc as bacc
import concourse.mybir as mybir
import concourse.tile as tile
from concourse import bass_utils
from concourse.masks import make_identity

P = 128
NC = 8
N = 100_000
NPAD = 100_352
SHARD = NPAD // NC  # 12544
TD = SHARD // P  # 98 dst tiles per core
NB = 4
BANK = NPAD // NB  # 25088
HC = SHARD // 2  # 6272 rows per shard chunk
HALF = NPAD // 2  # 50176
G = 7  # dst tiles per gather group
NG = TD // G  # 14 groups
D_IN = 256
H1 = 256
H2 = 128
f32 = mybir.dt.float32
f32r = mybir.dt.float32r
bf16 = mybir.dt.bfloat16
i16 = mybir.dt.int16

LAST_EXEC_NS = None
LAST_RESULT = None
LAST_TB = None


def _remap_ids(o):
    """old node id -> new node id (chunked shards for 2-piece AllGather)."""
    c = o // SHARD
    r = o % SHARD
    lo = r < HC
    return np.where(lo, c * HC + r, HALF + c * HC + (r - HC))


def _pack(edge_src, edge_dst, dinv_pad):
    """Sort edges (already in remapped id space) into per-core slot layouts.

    Layout: for grp in NG: for b in NB: for t in grp: T[t,b]*128 slots.
    Returns T table and per-core arrays (gidx wrapped, dstl cols,
    dinvslot cols).
    """
    percore = []
    runs_all = np.zeros((NC, TD, NB), dtype=np.int64)
    for c in range(NC):
        lo, hi = c * SHARD, (c + 1) * SHARD
        # new-id shard of core c is two chunks; map dst new-id -> local row
        # local row: chunk0 -> r, chunk1 -> HC + r
        d = edge_dst
        in0 = (d >= c * HC) & (d < (c + 1) * HC)
        in1 = (d >= HALF + c * HC) & (d < HALF + (c + 1) * HC)
        sel = in0 | in1
        s, d = edge_src[sel], edge_dst[sel]
        local = np.where(d < HALF, d - c * HC, HC + (d - (HALF + c * HC)))
        tile_id = local // P
        bank = s // BANK
        key = tile_id * NB + bank
        order = np.argsort(key, kind="stable")
        s, local, key = s[order], local[order], key[order]
        runs = np.bincount(key, minlength=TD * NB).reshape(TD, NB)
        runs_all[c] = runs
        percore.append((s, local, key, runs))
    T = (runs_all.max(axis=0) + P - 1) // P  # [TD, NB]
    T = np.maximum(T, 1)
    # slot base for each (t, b) in the global layout
    slot_base = np.zeros((TD, NB), dtype=np.int64)
    call_base = []  # (grp, b) -> slot offset, num_idxs
    off = 0
    for g in range(NG):
        for b in range(NB):
            call_off = off
            for t in range(g * G, (g + 1) * G):
                slot_base[t, b] = off
                off += T[t, b] * P
            call_base.append((call_off, off - call_off))
    nslots = off
    nsub = int(T.sum())
    # subtile column order mirrors the slot/call order: for grp: for b: for t
    # so each call's selector columns are contiguous
    subcol = np.zeros((TD, NB), dtype=np.int64)
    col = 0
    for g in range(NG):
        for b in range(NB):
            for t in range(g * G, (g + 1) * G):
                subcol[t, b] = col
                col += T[t, b]
    assert col == nsub

    cores = []
    for c in range(NC):
        s, local, key, runs = percore[c]
        first = np.zeros(TD * NB, dtype=np.int64)
        first[1:] = np.cumsum(runs.reshape(-1))[:-1]
        rank = np.arange(len(key)) - first[key]
        slot = slot_base[key // NB, key % NB] + rank
        gidx = np.zeros(nslots, dtype=np.int16)
        dstl = np.full(nslots, -1.0, dtype=np.float32)
        gidx[slot] = (s % BANK).astype(np.int16)
        dstl[slot] = (local % P).astype(np.float32)
        # wrap idxs per call
        parts = []
        for g in range(NG):
            for b in range(NB):
                co, n = call_base[g * NB + b]
                parts.append(gidx[co:co + n].reshape(n // 16, 16).T)
        wrapped = np.tile(np.concatenate(parts, axis=1), (8, 1))
        # dstl as [128, nsub] column table in subcol order (bf16-exact ints)
        dstl_cols = np.empty((P, nsub), dtype=np.float32)
        for t in range(TD):
            for b in range(NB):
                c0 = subcol[t, b]
                nT = T[t, b]
                blk = dstl[slot_base[t, b]:slot_base[t, b] + nT * P]
                dstl_cols[:, c0:c0 + nT] = blk.reshape(nT, P).T
        cores.append({
            "gidx": np.ascontiguousarray(wrapped),
            "dstl": dstl_cols.astype(ml_dtypes.bfloat16),
        })
    return T, call_base, subcol, nslots, nsub, cores


def _build(Tkey, T, call_base, subcol, nslots, nsub):
    TMAXC = max(n for _, n in call_base) // P  # max subtiles per call
    nc = bacc.Bacc(
        "TRN2",
        target_bir_lowering=False,
        debug=False,
        num_devices=NC,
        num_swdge_queues=4,
    )
    xs = nc.dram_tensor("xs", [NPAD, D_IN], bf16, kind="ExternalInput").ap()
    gidx = nc.dram_tensor("gidx", [P, nslots // 16], i16, kind="ExternalInput").ap()
    dstl_d = nc.dram_tensor("dstl", [P, nsub], bf16, kind="ExternalInput").ap()
    w1 = nc.dram_tensor("w1", [D_IN, H1], f32, kind="ExternalInput").ap()
    w2 = nc.dram_tensor("w2", [H1, H2], f32, kind="ExternalInput").ap()
    b1c = nc.dram_tensor("b1c", [P, H1 // P], f32, kind="ExternalInput").ap()
    b2r = nc.dram_tensor("b2r", [P, H2], f32, kind="ExternalInput").ap()
    dinv_d = nc.dram_tensor("dinv_d", [P, TD], f32, kind="ExternalInput").ap()
    out = nc.dram_tensor("out", [SHARD, H2], f32, kind="ExternalOutput").ap()

    qn = [0]

    def next_q():
        qn[0] = (qn[0] + 1) % 4
        return qn[0]

    with tile.TileContext(nc) as tc:
        with (
            tc.tile_pool(name="const", bufs=1) as cp,
            tc.tile_pool(name="msg", bufs=6) as mp,
            tc.tile_pool(name="sel", bufs=4) as sp,
            tc.tile_pool(name="work", bufs=2) as wp,
            tc.tile_pool(name="grp", bufs=2) as gp,
            tc.tile_pool(name="psy", bufs=2, space="PSUM") as psy,
            tc.tile_pool(name="pst", bufs=2, space="PSUM") as pst,
            tc.tile_pool(name="psh", bufs=2, space="PSUM") as psh,
            tc.tile_pool(name="dram", bufs=1, space="DRAM") as dp,
        ):
            # ---- constants ----
            # q-major iota: col (q*TMAXC + s) = q, for batched selector builds
            iota_i = cp.tile([P, P * TMAXC], mybir.dt.int32)
            nc.gpsimd.iota(iota_i[:], pattern=[[1, P], [0, TMAXC]], base=0,
                           channel_multiplier=0)
            iota_q = cp.tile([P, P * TMAXC], bf16)
            nc.vector.tensor_copy(iota_q[:], iota_i[:])
            ident = cp.tile([P, P], f32)
            make_identity(nc, ident[:])

            gidx_t = cp.tile([P, nslots // 16], i16)
            nc.sync.dma_start(gidx_t[:], gidx[:, :])
            dstl_t = cp.tile([P, nsub], bf16)
            nc.sync.dma_start(dstl_t[:], dstl_d[:, :])
            b1_t = cp.tile([P, H1 // P], f32)
            nc.sync.dma_start(b1_t[:], b1c[:, :])
            b2_t = cp.tile([P, H2], f32)
            nc.sync.dma_start(b2_t[:], b2r[:, :])
            dinv_t = cp.tile([P, TD], f32)
            nc.sync.dma_start(dinv_t[:], dinv_d[:, :])
            w1_t = [cp.tile([P, H1], f32r, tag=f"w1_{k}", name=f"w1_{k}") for k in range(2)]
            for k in range(2):
                nc.gpsimd.dma_start(w1_t[k][:], w1[k * P:(k + 1) * P, :])
            w2_t = [cp.tile([P, H2], f32r, tag=f"w2_{k}", name=f"w2_{k}") for k in range(2)]
            for k in range(2):
                nc.gpsimd.dma_start(w2_t[k][:], w2[k * P:(k + 1) * P, :])

            h2sA = dp.tile([HC, H2], bf16)
            h2sB = dp.tile([HC, H2], bf16)
            h2A = dp.tile([HALF, H2], bf16)
            h2B = dp.tile([HALF, H2], bf16)

            def gather_call(g, b, elem, src_ap):
                co, n = call_base[g * NB + b]
                m = mp.tile([P, TMAXC, D_IN], bf16, tag="msg", name="m")
                nsubt = n // P
                # view the first nsubt*elem columns as [P, nsubt, elem]
                flat = m[:].rearrange("p t d -> p (t d)")
                view = flat[:, :nsubt * elem].rearrange("p (t d) -> p t d", t=nsubt)
                nc.gpsimd.dma_gather(
                    out_ap=view,
                    in_ap=src_ap,
                    idxs_ap=gidx_t[:, co // 16:(co + n) // 16],
                    num_idxs=n,
                    num_idxs_reg=n,
                    elem_size=elem,
                    single_packet=False,
                    queue_num=next_q(),
                )
                return m, nsubt

            def build_sel(g, b):
                """One DVE op builds all subtile selectors of call (g,b).

                Output layout [128 slots_p, 128 q, TMAXC s]: col q*TMAXC+s =
                (dstl[p, cb+s] == q).
                """
                cb = int(subcol[g * G, b])
                S = call_base[g * NB + b][1] // P
                sel = sp.tile([P, P, TMAXC], bf16, tag="sel", name="sel")
                in0 = iota_q[:].rearrange("p (q s) -> p q s", q=P)[:, :, :S]
                d = dstl_t[:, cb:cb + S]
                in1 = bass.AP(
                    d.tensor, d.offset, [d.ap[0], [0, P], [d.ap[1][0], S]]
                )
                nc.vector.tensor_tensor(
                    out=sel[:, :, :S],
                    in0=in0,
                    in1=in1,
                    op=mybir.AluOpType.is_equal,
                )
                return sel

            def segsum(g, t, msgs, sels, elem, acc):
                """acc[dst,elem] += sum over subtiles sel^T @ msg."""
                K = int(T[t].sum())
                k = 0
                for b in range(NB):
                    m = msgs[b]
                    flat = m[:].rearrange("p t d -> p (t d)")
                    # subtile offset of tile t within call (g,b)
                    sb = int(subcol[t, b] - subcol[g * G, b])
                    for j in range(int(T[t, b])):
                        w = sb + j
                        rhs = flat[:, w * elem:(w + 1) * elem]
                        nc.tensor.matmul(
                            acc[:],
                            lhsT=sels[b][:, :, w],
                            rhs=rhs,
                            start=(k == 0),
                            stop=(k == K - 1),
                        )
                        k += 1

            # ================= layer 1 =================
            for g in range(NG):
                msgs = []
                sels = []
                for b in range(NB):
                    m, _ = gather_call(g, b, D_IN, xs[b * BANK:(b + 1) * BANK, :])
                    msgs.append(m)
                    sels.append(build_sel(g, b))
                yT = [gp.tile([P, G * P], f32r, tag=f"yT{h}", name=f"yT{h}") for h in range(2)]
                for ti in range(G):
                    t = g * G + ti
                    acc = psy.tile([P, D_IN], f32, tag="acc1")
                    segsum(g, t, msgs, sels, D_IN, acc)
                    y_sb = wp.tile([P, D_IN], f32, tag="ysb")
                    nc.vector.tensor_scalar(
                        out=y_sb[:],
                        in0=acc[:],
                        scalar1=dinv_t[:, t:t + 1],
                        scalar2=None,
                        op0=mybir.AluOpType.mult,
                    )
                    for h in range(2):
                        tp = pst.tile([P, P], f32, tag="tp")
                        nc.tensor.transpose(
                            out=tp[:], in_=y_sb[:, h * P:(h + 1) * P], identity=ident[:]
                        )
                        nc.vector.tensor_copy(yT[h][:, ti * P:(ti + 1) * P], tp[:])
                # dense, in PSUM-bank-sized column chunks (<=512 f32)
                nn = G * P
                chunks = [(0, 512), (512, nn)]
                rT = [gp.tile([P, G * P], f32r, tag=f"rT{o}", name=f"rT{o}") for o in range(2)]
                h2T_sb = wp.tile([P, G * P], f32, tag="h2T")
                for c0, c1 in chunks:
                    cw = c1 - c0
                    for o in range(2):
                        ph1 = psh.tile([P, 512], f32, tag="ph1")
                        for k in range(2):
                            nc.tensor.matmul(
                                ph1[:, :cw],
                                lhsT=w1_t[k][:, o * P:(o + 1) * P],
                                rhs=yT[k][:, c0:c1],
                                start=(k == 0),
                                stop=(k == 1),
                            )
                        nc.scalar.activation(
                            out=rT[o][:, c0:c1],
                            in_=ph1[:, :cw],
                            func=mybir.ActivationFunctionType.Relu,
                            bias=b1_t[:, o:o + 1],
                            scale=1.0,
                        )
                    ph2 = psh.tile([P, 512], f32, tag="ph1")
                    for k in range(2):
                        nc.tensor.matmul(
                            ph2[:, :cw],
                            lhsT=w2_t[k][:, :],
                            rhs=rT[k][:, c0:c1],
                            start=(k == 0),
                            stop=(k == 1),
                        )
                    nc.vector.tensor_copy(h2T_sb[:, c0:c1], ph2[:, :cw])
                h2_sb = wp.tile([P, G, P], bf16, tag="h2sb")
                for ti in range(G):
                    t = g * G + ti
                    tp2 = pst.tile([P, P], f32, tag="tp")
                    nc.tensor.transpose(
                        out=tp2[:], in_=h2T_sb[:, ti * P:(ti + 1) * P], identity=ident[:]
                    )
                    nc.vector.tensor_scalar(
                        out=h2_sb[:, ti, :],
                        in0=tp2[:],
                        scalar1=dinv_t[:, t:t + 1],
                        scalar2=None,
                        op0=mybir.AluOpType.mult,
                    )
                if g < NG // 2:
                    dst_rows = h2sA[g * G * P:(g * G + G) * P, :].rearrange(
                        "(t p) f -> p t f", p=P)
                else:
                    g2 = g - NG // 2
                    dst_rows = h2sB[g2 * G * P:(g2 * G + G) * P, :].rearrange(
                        "(t p) f -> p t f", p=P)
                nc.sync.dma_start(dst_rows, h2_sb[:, :, :])

                # chunk-0 AllGather once the first half of the shard is done
                if g == NG // 2 - 1:
                    nc.gpsimd.collective_compute(
                        "AllGather",
                        mybir.AluOpType.bypass,
                        ins=[h2sA.opt()],
                        outs=[h2A.opt()],
                        replica_groups=[list(range(NC))],
                    )
            nc.gpsimd.collective_compute(
                "AllGather",
                mybir.AluOpType.bypass,
                ins=[h2sB.opt()],
                outs=[h2B.opt()],
                replica_groups=[list(range(NC))],
            )

            # ================= layer 2 =================
            for g in range(NG):
                msgs = []
                sels = []
                for b in range(NB):
                    if b < 2:
                        src = h2A[b * BANK:(b + 1) * BANK, :]
                    else:
                        src = h2B[(b - 2) * BANK:(b - 1) * BANK, :]
                    m, _ = gather_call(g, b, H2, src)
                    msgs.append(m)
                    sels.append(build_sel(g, b))
                for ti in range(G):
                    t = g * G + ti
                    acc2 = pst.tile([P, H2], f32, tag="tp")
                    segsum(g, t, msgs, sels, H2, acc2)
                    o_tmp = wp.tile([P, H2], f32, tag="otmp")
                    nc.vector.tensor_scalar(
                        out=o_tmp[:],
                        in0=acc2[:],
                        scalar1=dinv_t[:, t:t + 1],
                        scalar2=None,
                        op0=mybir.AluOpType.mult,
                    )
                    o_sb = wp.tile([P, H2], f32, tag="osb")
                    nc.vector.tensor_tensor(
                        out=o_sb[:],
                        in0=o_tmp[:],
                        in1=b2_t[:],
                        op=mybir.AluOpType.add,
                    )
                    nc.sync.dma_start(out[t * P:(t + 1) * P, :], o_sb[:])

    nc.compile()
    return nc


_CACHED = {}


def kernel(x, W1, b1, W2, b2, edge_index):
    global LAST_EXEC_NS, LAST_RESULT, LAST_TB
    x = np.asarray(x, dtype=np.float32)
    W1 = np.asarray(W1, dtype=np.float32)
    b1 = np.asarray(b1, dtype=np.float32)
    W2 = np.asarray(W2, dtype=np.float32)
    b2 = np.asarray(b2, dtype=np.float32)
    ei = np.asarray(edge_index)
    src = ei[0].astype(np.int64)
    dst = ei[1].astype(np.int64)
    n = x.shape[0]
    loop = np.arange(n, dtype=np.int64)
    src_f = np.concatenate([src, loop])
    dst_f = np.concatenate([dst, loop])
    deg = np.bincount(dst_f, minlength=n).astype(np.float32)
    dinv = np.where(deg > 0, 1.0 / np.sqrt(deg), 0.0).astype(np.float32)

    # remap ids to chunked-shard space
    src_r = _remap_ids(src_f)
    dst_r = _remap_ids(dst_f)
    newpos = _remap_ids(np.arange(NPAD, dtype=np.int64))  # old -> new
    dinv_pad = np.zeros(NPAD, dtype=np.float32)
    dinv_pad[:n] = dinv
    dinv_new = np.zeros(NPAD, dtype=np.float32)
    dinv_new[newpos] = dinv_pad

    xs = np.zeros((NPAD, D_IN), dtype=ml_dtypes.bfloat16)
    xs[newpos[:n]] = (x * dinv[:, None]).astype(ml_dtypes.bfloat16)

    T, call_base, subcol, nslots, nsub, cores = _pack(src_r, dst_r, dinv_new)
    LAST_TB = (nslots, nsub)

    key = hash((T.tobytes(), G))
    if key not in _CACHED:
        _CACHED[key] = _build(key, T, call_base, subcol, nslots, nsub)
    ncobj = _CACHED[key]

    b1c = b1.reshape(H1 // P, P).T.copy()
    b2r = np.tile(b2.reshape(1, H2), (P, 1)).astype(np.float32)
    in_maps = []
    for c in range(NC):
        # dinv per dst tile, new-id order: tiles 0..48 chunk0, 49..97 chunk1
        rows = np.concatenate([
            dinv_new[c * HC:(c + 1) * HC],
            dinv_new[HALF + c * HC:HALF + (c + 1) * HC],
        ])
        dinv_d = rows.reshape(TD, P).T.copy()
        in_maps.append({
            "xs": xs,
            "gidx": cores[c]["gidx"],
            "dstl": cores[c]["dstl"],
            "w1": W1,
            "w2": W2,
            "b1c": b1c,
            "b2r": b2r,
            "dinv_d": dinv_d,
        })

    trace = os.environ.get("KERNEL_TRACE", "0") == "1"
    if trace:
        try:
            import profhook

            profhook.install()
        except Exception:
            trace = False
    res = bass_utils.run_bass_kernel_spmd(
        ncobj, in_maps, core_ids=list(range(NC)), trace=trace
    )
    LAST_EXEC_NS = res.exec_time_ns
    LAST_RESULT = res
    out_new = np.zeros((NPAD, H2), dtype=np.float32)
    for c in range(NC):
        r = np.asarray(res.results[c]["out"], dtype=np.float32)
        out_new[c * HC:(c + 1) * HC] = r[:HC]
        out_new[HALF + c * HC:HALF + (c + 1) * HC] = r[HC:]
    out = out_new[newpos[:n]]
    return out.astype(np.float32)


# revision 32
# speedup vs baseline: 1.2271x; 1.1877x over previous
"""GCN 2-layer encoder on 8 TRN2 NeuronCores.

Strategy (graph/data parallel, aggregate-first form):
  out = A_hat @ (relu((A_hat @ x) @ W1 + b1) @ W2) + b2
where A_hat = D^-1/2 (A + I) D^-1/2.  Since aggregation is linear it
commutes with the dense layer:  A_hat (x W1) == (A_hat x) W1.

Per core (nodes sharded 8 x 12544 padded rows):
  1. gather x_scaled[src] rows (x pre-scaled by dinv on host) with
     dma_gather (int16 indices, 4 banks of 25088 rows, 4 SWDGE queues),
     one-hot segment-sum matmul per 128-node dst tile (f32r),
     epilogue scales by dinv[dst] -> y = A_hat x
  2. yT via PE transpose; h1T = W1^T yT; relu+bias (ACT);
     h2T = W2^T rT; transpose back -> h2 rows, scaled by dinv[node]
  3. AllGather h2 shards -> full h2 table (51MB DRAM per core)
  4. same gather/segsum in bf16 over h2 -> + b2 -> output shard
Host assembles the 8 shards.
"""
import sys

sys.path.insert(0, "/opt/trn_rl_repo")
import os
import numpy as np
import ml_dtypes

import concourse.bass as bass
import concourse.bacc as bacc
import concourse.mybir as mybir
import concourse.tile as tile
from concourse import bass_utils
from concourse.masks import make_identity

P = 128
NC = 8
N = 100_000
NPAD = 100_352  # 8 * 12544
SHARD = NPAD // NC  # 12544
TD = SHARD // P  # 98 dst tiles per core
NB = 4  # index banks (int16 range)
BANK = NPAD // NB  # 25088
HC = SHARD // 2  # 6272 rows per shard chunk (49 tiles)
HALF = NPAD // 2  # 50176 = 2 * BANK
TA = 49  # tiles in chunk A


def _remap_ids(o):
    """old node id -> new node id (chunked shards for 2-piece AllGather)."""
    c = o // SHARD
    r = o % SHARD
    lo = r < HC
    return np.where(lo, c * HC + r, HALF + c * HC + (r - HC))
D_IN = 256
H1 = 256
H2 = 128
f32 = mybir.dt.float32
f32r = mybir.dt.float32r
bf16 = mybir.dt.bfloat16
i16 = mybir.dt.int16

LAST_EXEC_NS = None
LAST_RESULT = None
LAST_TB = None


def _pack(edge_src, edge_dst):
    """Sort/pad edges into per-core (group, bank, tile, subtile) slots.

    Group-major layout so one dma_gather covers a whole group's bank runs.
    Returns T_b and per-core dicts of index/selector arrays.
    """
    GRP = 4
    ngrp = (TD + GRP - 1) // GRP
    gsizes = [min(GRP, TD - g * GRP) for g in range(ngrp)]
    cores = []
    maxrun = 0
    percore = []
    for c in range(NC):
        d = edge_dst
        in0 = (d >= c * HC) & (d < (c + 1) * HC)
        in1 = (d >= HALF + c * HC) & (d < HALF + (c + 1) * HC)
        sel = in0 | in1
        s, d = edge_src[sel], edge_dst[sel]
        local = np.where(d < HALF, d - c * HC, HC + (d - (HALF + c * HC)))
        tile_id = local // P
        bank = s // BANK
        key = tile_id * NB + bank
        order = np.argsort(key, kind="stable")
        s, local, key = s[order], local[order], key[order]
        runs = np.bincount(key, minlength=TD * NB)
        maxrun = max(maxrun, int(runs.max()))
        percore.append((s, local, key, runs))
    T_b = (maxrun + P - 1) // P
    T_e = NB * T_b
    # slot base of each (tile, bank) run in group-major order
    grp_of = np.arange(TD) // GRP
    di_of = np.arange(TD) % GRP
    gs_of = np.array([gsizes[g] for g in grp_of])
    grp_base = np.zeros(ngrp, dtype=np.int64)
    for g in range(1, ngrp):
        grp_base[g] = grp_base[g - 1] + NB * gsizes[g - 1] * T_b * P
    nslots = int(grp_base[-1] + NB * gsizes[-1] * T_b * P)
    run_base = np.zeros((TD, NB), dtype=np.int64)
    for d in range(TD):
        g, di, gs = grp_of[d], di_of[d], gs_of[d]
        for b in range(NB):
            run_base[d, b] = grp_base[g] + (b * gs + di) * (T_b * P)
    for c in range(NC):
        s, local, key, runs = percore[c]
        gidx = np.zeros(nslots, dtype=np.int16)
        dstl = np.full(nslots, -1.0, dtype=np.float32)
        first = np.zeros(TD * NB, dtype=np.int64)
        first[1:] = np.cumsum(runs)[:-1]
        rank = np.arange(len(key)) - first[key]
        slot = run_base[key // NB, key % NB] + rank
        gidx[slot] = (s % BANK).astype(np.int16)
        dstl[slot] = (local % P).astype(np.float32)
        # wrap for dma_gather: per call (grp, bank) = gs*T_b*128 linear idxs
        wrapped_parts = []
        for g in range(ngrp):
            gs = gsizes[g]
            w = gs * T_b * P
            for b in range(NB):
                a = gidx[grp_base[g] + b * w : grp_base[g] + (b + 1) * w]
                wrapped_parts.append(a.reshape(w // 16, 16).T)
        wrapped16 = np.concatenate(wrapped_parts, axis=1)
        wrapped = np.tile(wrapped16, (8, 1))
        # selector cols: [128, TD*T_e], col = d*T_e + b*T_b + j, row q
        dstl_cols = np.zeros((P, TD * T_e), dtype=np.float32)
        for d2 in range(TD):
            g, di, gs = grp_of[d2], di_of[d2], gs_of[d2]
            for b in range(NB):
                blk = dstl[run_base[d2, b] : run_base[d2, b] + T_b * P]
                dstl_cols[:, d2 * T_e + b * T_b : d2 * T_e + (b + 1) * T_b] = (
                    blk.reshape(T_b, P).T
                )
        cores.append(
            {
                "gidx": np.ascontiguousarray(wrapped),
                "dstl_bf": dstl_cols.astype(ml_dtypes.bfloat16),
            }
        )
    return T_b, cores


def _build(T_b):
    T_e = NB * T_b
    nc = bacc.Bacc(
        "TRN2",
        target_bir_lowering=False,
        debug=False,
        num_devices=NC,
        num_swdge_queues=4,
    )
    xs = nc.dram_tensor("xs", [NPAD, D_IN], bf16, kind="ExternalInput").ap()
    gidx = nc.dram_tensor(
        "gidx", [P, TD * NB * T_b * 8], i16, kind="ExternalInput"
    ).ap()
    dstl_bf = nc.dram_tensor(
        "dstl_bf", [P, TD * T_e], bf16, kind="ExternalInput"
    ).ap()
    w1 = nc.dram_tensor("w1", [D_IN, H1], f32, kind="ExternalInput").ap()
    w2 = nc.dram_tensor("w2", [H1, H2], f32, kind="ExternalInput").ap()
    b1c = nc.dram_tensor("b1c", [P, H1 // P], f32, kind="ExternalInput").ap()
    b2c = nc.dram_tensor("b2c", [P, 1], f32, kind="ExternalInput").ap()
    dinv_d = nc.dram_tensor("dinv_d", [P, TD], f32, kind="ExternalInput").ap()
    out = nc.dram_tensor("out", [SHARD, H2], f32, kind="ExternalOutput").ap()

    qn = [0]

    def next_q():
        qn[0] = (qn[0] + 1) % 4
        return qn[0]

    with tile.TileContext(nc) as tc:
        with (
            tc.tile_pool(name="const", bufs=1) as cp,
            tc.tile_pool(name="msg", bufs=16) as mp,
            tc.tile_pool(name="sp", bufs=4) as spp,
            tc.tile_pool(name="work", bufs=3) as wp,
            tc.tile_pool(name="grp", bufs=2) as gp,
            tc.tile_pool(name="psy", bufs=4, space="PSUM") as psy,
            tc.tile_pool(name="pst", bufs=2, space="PSUM") as pst,
            tc.tile_pool(name="psh", bufs=2, space="PSUM") as psh,
            tc.tile_pool(name="dram", bufs=1, space="DRAM") as dp,
        ):
            # ---- constants ----
            iota_i = cp.tile([P, T_e * P], mybir.dt.int32)
            nc.gpsimd.iota(iota_i[:], pattern=[[0, T_e], [1, P]], base=0, channel_multiplier=0)
            iota_bf = cp.tile([P, T_e * P], bf16)
            nc.vector.tensor_copy(iota_bf[:], iota_i[:])
            ident = cp.tile([P, P], f32)
            make_identity(nc, ident[:])

            gidx_t = cp.tile([P, TD * NB * T_b * 8], i16)
            nc.sync.dma_start(gidx_t[:], gidx[:, :])
            dstlb_t = cp.tile([P, TD * T_e], bf16)
            nc.sync.dma_start(dstlb_t[:], dstl_bf[:, :])
            b1_t = cp.tile([P, H1 // P], f32)
            nc.sync.dma_start(b1_t[:], b1c[:, :])
            b2_t = cp.tile([P, 1], f32)
            nc.sync.dma_start(b2_t[:], b2c[:, :])
            dinv_t = cp.tile([P, TD], f32)
            nc.sync.dma_start(dinv_t[:], dinv_d[:, :])
            w1_t = [cp.tile([P, H1], f32r, tag=f"w1_{k}", name=f"w1_{k}") for k in range(2)]
            for k in range(2):
                nc.gpsimd.dma_start(w1_t[k][:], w1[k * P : (k + 1) * P, :])
            w2_t = [cp.tile([P, H2], f32r, tag=f"w2_{k}", name=f"w2_{k}") for k in range(2)]
            for k in range(2):
                nc.gpsimd.dma_start(w2_t[k][:], w2[k * P : (k + 1) * P, :])

            h2sA = dp.tile([TA * P, H2], bf16)
            h2sB = dp.tile([TA * P, H2], bf16)
            h2A = dp.tile([HALF, H2], bf16)
            h2B = dp.tile([HALF, H2], bf16)

            def bcast(ap_tile, d0, n_t):
                a = ap_tile[:, d0 : d0 + n_t]
                return bass.AP(
                    a.tensor, a.offset, [a.ap[0], [a.ap[1][0], n_t], [0, P]]
                )

            # ================= layer 1 + dense =================
            GRP = 4
            ngrp = (TD + GRP - 1) // GRP
            gsizes = [min(GRP, TD - g * GRP) for g in range(ngrp)]
            col_base = [0]
            for g in range(ngrp):
                for b in range(NB):
                    col_base.append(col_base[-1] + gsizes[g] * T_b * 8)
            for g in range(ngrp):
                gs = gsizes[g]
                dlist = list(range(g * GRP, g * GRP + gs))
                yT = [gp.tile([P, GRP * P], f32r, tag=f"yT{h}", name=f"yT{h}") for h in range(2)]
                for di, d in enumerate(dlist):
                    msgs = []
                    for b in range(NB):
                        m = mp.tile([P, T_b, D_IN], bf16, tag="msg1", name="m1")
                        cb = col_base[g * NB + b] + di * T_b * 8
                        nc.gpsimd.dma_gather(
                            out_ap=m[:],
                            in_ap=xs[b * BANK : (b + 1) * BANK, :],
                            idxs_ap=gidx_t[:, cb : cb + T_b * 8],
                            num_idxs=T_b * P,
                            num_idxs_reg=T_b * P,
                            elem_size=D_IN,
                            single_packet=False,
                            queue_num=next_q(),
                        )
                        msgs.append(m)
                    sp = spp.tile([P, T_e * P], bf16, tag="sp1")
                    nc.vector.tensor_tensor(
                        out=sp[:],
                        in0=iota_bf[:].rearrange("p (t q) -> p t q", t=T_e),
                        in1=bcast(dstlb_t, d * T_e, T_e),
                        op=mybir.AluOpType.is_equal,
                    )
                    acc = psy.tile([P, D_IN], f32, tag="acc1")
                    for b in range(NB):
                        for j in range(T_b):
                            t = b * T_b + j
                            nc.tensor.matmul(
                                acc[:],
                                lhsT=sp[:, t * P : (t + 1) * P],
                                rhs=msgs[b][:, j, :],
                                start=(t == 0),
                                stop=(t == T_e - 1),
                            )
                    y_sb = wp.tile([P, D_IN], f32, tag="ysb")
                    nc.vector.tensor_scalar(
                        out=y_sb[:],
                        in0=acc[:],
                        scalar1=dinv_t[:, d : d + 1],
                        scalar2=None,
                        op0=mybir.AluOpType.mult,
                    )
                    for h in range(2):
                        tp = pst.tile([P, P], f32, tag="tp")
                        nc.tensor.transpose(
                            out=tp[:], in_=y_sb[:, h * P : (h + 1) * P], identity=ident[:]
                        )
                        nc.vector.tensor_copy(yT[h][:, di * P : (di + 1) * P], tp[:])
                # dense: h1T = W1^T yT ; rT = relu(h1T + b1); h2T = W2^T rT
                nn = gs * P
                rT = [gp.tile([P, GRP * P], f32r, tag=f"rT{o}", name=f"rT{o}") for o in range(2)]
                for o in range(2):
                    ph1 = psh.tile([P, GRP * P], f32, tag="ph1")
                    for k in range(2):
                        nc.tensor.matmul(
                            ph1[:, :nn],
                            lhsT=w1_t[k][:, o * P : (o + 1) * P],
                            rhs=yT[k][:, :nn],
                            start=(k == 0),
                            stop=(k == 1),
                        )
                    nc.scalar.activation(
                        out=rT[o][:, :nn],
                        in_=ph1[:, :nn],
                        func=mybir.ActivationFunctionType.Relu,
                        bias=b1_t[:, o : o + 1],
                        scale=1.0,
                    )
                ph2 = psh.tile([P, GRP * P], f32, tag="ph1")
                for k in range(2):
                    nc.tensor.matmul(
                        ph2[:, :nn],
                        lhsT=w2_t[k][:, :],
                        rhs=rT[k][:, :nn],
                        start=(k == 0),
                        stop=(k == 1),
                    )
                h2T_sb = wp.tile([P, GRP * P], f32, tag="h2T")
                nc.vector.tensor_copy(h2T_sb[:, :nn], ph2[:, :nn])
                h2_sb = wp.tile([P, GRP, P], bf16, tag="h2sb")
                for qi, d in enumerate(dlist):
                    tp2 = pst.tile([P, P], f32, tag="tp")
                    nc.tensor.transpose(
                        out=tp2[:], in_=h2T_sb[:, qi * P : (qi + 1) * P], identity=ident[:]
                    )
                    nc.vector.tensor_scalar(
                        out=h2_sb[:, qi, :],
                        in0=tp2[:],
                        scalar1=dinv_t[:, d : d + 1],
                        scalar2=None,
                        op0=mybir.AluOpType.mult,
                    )
                t0 = dlist[0]
                t1 = dlist[0] + gs
                if t1 <= TA:
                    dst_rows = h2sA[t0 * P:t1 * P, :].rearrange(
                        "(t p) f -> p t f", p=P)
                    nc.sync.dma_start(dst_rows, h2_sb[:, :gs, :])
                elif t0 >= TA:
                    dst_rows = h2sB[(t0 - TA) * P:(t1 - TA) * P, :].rearrange(
                        "(t p) f -> p t f", p=P)
                    nc.sync.dma_start(dst_rows, h2_sb[:, :gs, :])
                else:
                    na = TA - t0
                    ra = h2sA[t0 * P:TA * P, :].rearrange("(t p) f -> p t f", p=P)
                    nc.sync.dma_start(ra, h2_sb[:, :na, :])
                    rb = h2sB[0:(t1 - TA) * P, :].rearrange("(t p) f -> p t f", p=P)
                    nc.sync.dma_start(rb, h2_sb[:, na:gs, :])
                # chunk-A AllGather as soon as tiles 0..TA-1 are written,
                # overlapping the rest of layer 1
                if dlist[0] <= TA - 1 < dlist[0] + gs:
                    nc.gpsimd.collective_compute(
                        "AllGather",
                        mybir.AluOpType.bypass,
                        ins=[h2sA.opt()],
                        outs=[h2A.opt()],
                        replica_groups=[list(range(NC))],
                    )

            # ================= exchange (chunk B) =================
            nc.gpsimd.collective_compute(
                "AllGather",
                mybir.AluOpType.bypass,
                ins=[h2sB.opt()],
                outs=[h2B.opt()],
                replica_groups=[list(range(NC))],
            )

            # ================= layer 2 =================
            for g in range(ngrp):
                gs = gsizes[g]
                dlist = list(range(g * GRP, g * GRP + gs))
                for di, d in enumerate(dlist):
                    msgs = []
                    for b in range(NB):
                        m = mp.tile([P, T_b, H2], bf16, tag="msg2", name="m2")
                        cb = col_base[g * NB + b] + di * T_b * 8
                        h2src = (h2A[b * BANK:(b + 1) * BANK, :] if b < 2
                                 else h2B[(b - 2) * BANK:(b - 1) * BANK, :])
                        nc.gpsimd.dma_gather(
                            out_ap=m[:],
                            in_ap=h2src,
                            idxs_ap=gidx_t[:, cb : cb + T_b * 8],
                            num_idxs=T_b * P,
                            num_idxs_reg=T_b * P,
                            elem_size=H2,
                            single_packet=False,
                            queue_num=next_q(),
                        )
                        msgs.append(m)
                    sp2 = spp.tile([P, T_e * P], bf16, tag="sp2")
                    nc.vector.tensor_tensor(
                        out=sp2[:],
                        in0=iota_bf[:].rearrange("p (t q) -> p t q", t=T_e),
                        in1=bcast(dstlb_t, d * T_e, T_e),
                        op=mybir.AluOpType.is_equal,
                    )
                    acc2 = psy.tile([P, H2], f32, tag="acc1")
                    for b in range(NB):
                        for j in range(T_b):
                            t = b * T_b + j
                            nc.tensor.matmul(
                                acc2[:],
                                lhsT=sp2[:, t * P : (t + 1) * P],
                                rhs=msgs[b][:, j, :],
                                start=(t == 0),
                                stop=(t == T_e - 1),
                            )
                    o_sb = wp.tile([P, H2], f32, tag="osb")
                    nc.vector.tensor_scalar(
                        out=o_sb[:],
                        in0=acc2[:],
                        scalar1=dinv_t[:, d : d + 1],
                        scalar2=b2_t[:, :1],
                        op0=mybir.AluOpType.mult,
                        op1=mybir.AluOpType.add,
                    )
                    nc.sync.dma_start(out[d * P : (d + 1) * P, :], o_sb[:])

    nc.compile()
    return nc


_CACHED = {}


def kernel(x, W1, b1, W2, b2, edge_index):
    global LAST_EXEC_NS, LAST_RESULT, LAST_TB
    x = np.asarray(x, dtype=np.float32)
    W1 = np.asarray(W1, dtype=np.float32)
    b1 = np.asarray(b1, dtype=np.float32)
    W2 = np.asarray(W2, dtype=np.float32)
    b2 = np.asarray(b2, dtype=np.float32)
    ei = np.asarray(edge_index)
    src = ei[0].astype(np.int64)
    dst = ei[1].astype(np.int64)
    n = x.shape[0]
    # self loops
    loop = np.arange(n, dtype=np.int64)
    src_f = np.concatenate([src, loop])
    dst_f = np.concatenate([dst, loop])
    deg = np.bincount(dst_f, minlength=n).astype(np.float32)
    dinv = np.where(deg > 0, 1.0 / np.sqrt(deg), 0.0).astype(np.float32)

    # remap ids to chunked-shard space (2-piece AllGather)
    src_r = _remap_ids(src_f)
    dst_r = _remap_ids(dst_f)
    newpos = _remap_ids(np.arange(NPAD, dtype=np.int64))
    dinv_pad0 = np.zeros(NPAD, dtype=np.float32)
    dinv_pad0[:n] = dinv
    dinv_pad = np.zeros(NPAD, dtype=np.float32)
    dinv_pad[newpos] = dinv_pad0

    xs = np.zeros((NPAD, D_IN), dtype=ml_dtypes.bfloat16)
    xs[newpos[:n]] = (x * dinv[:, None]).astype(ml_dtypes.bfloat16)

    T_b, cores = _pack(src_r, dst_r)
    global LAST_TB
    LAST_TB = T_b

    key = T_b
    if key not in _CACHED:
        _CACHED[key] = _build(T_b)
    ncobj = _CACHED[key]

    b1c = b1.reshape(H1 // P, P).T.copy()
    b2c = b2.reshape(1, P).T.copy()
    in_maps = []
    for c in range(NC):
        rows = np.concatenate([
            dinv_pad[c * HC:(c + 1) * HC],
            dinv_pad[HALF + c * HC:HALF + (c + 1) * HC],
        ])
        dinv_d = rows.reshape(TD, P).T.copy()
        in_maps.append(
            {
                "xs": xs,
                "gidx": cores[c]["gidx"],
                "dstl_bf": cores[c]["dstl_bf"],
                "w1": W1,
                "w2": W2,
                "b1c": b1c,
                "b2c": b2c,
                "dinv_d": dinv_d,
            }
        )

    trace = os.environ.get("KERNEL_TRACE", "0") == "1"
    if trace:
        try:
            import profhook

            profhook.install()
        except Exception:
            trace = False
    res = bass_utils.run_bass_kernel_spmd(
        ncobj, in_maps, core_ids=list(range(NC)), trace=trace
    )
    LAST_EXEC_NS = res.exec_time_ns
    global LAST_RESULT
    LAST_RESULT = res
    out_new = np.zeros((NPAD, H2), dtype=np.float32)
    for c in range(NC):
        r = np.asarray(res.results[c]["out"], dtype=np.float32)
        out_new[c * HC:(c + 1) * HC] = r[:HC]
        out_new[HALF + c * HC:HALF + (c + 1) * HC] = r[HC:]
    return out_new[newpos[:n]].astype(np.float32)

